# revision 20
# baseline (speedup 1.0000x reference)
"""Trainium2 Bass kernel for DecoupledMVRowSelfAttnProcessor.

Sharding: tensor-parallel over heads — 8 cores x 2 heads each.
Each core computes, for its 128-wide head slice, all three branches
(self-attn, ref cross-attn, multi-view row attn) and fused
out-projections; host sums the 8 fp16 partials + residual + biases.

v2: software-pipelined emission — attention of batch i overlaps the
projections of batch i+1; the multi-view attention of group g runs
interleaved into batch 6g+5's attention; the out-projections of
group g are deferred into group g+1's batches (last group: final
batch + epilogue).  fp8 q/k (FWL LDWEIGHTS), scores exp'd to fp8 and
consumed by fp8 DoubleRow PV matmuls over kt-pairs (v stationary
padded to 80 cols with a ones column at 64 for the softmax
denominator).  PSUM: scores pool (4 banks) + PV accumulators
(2 banks) + projection/outproj/mv scratch (2 banks).
Norm: den row copied PSUM->SBUF before reciprocal_approx_fast
(the custom DVE op reads garbage from PSUM).
"""
import sys

for _p in ('/opt/trn_rl_repo',):
    if _p not in sys.path:
        sys.path.insert(0, _p)

import numpy as np
import ml_dtypes

# ---- problem constants (hardcoded per contest rules) ----
B, S, C = 12, 1024, 1024
HEADS, D = 16, 64
NV, IH, IW = 6, 32, 32
T = B * S                # 12288 tokens
HL = 2                   # heads per core
D2 = HL * D              # 128: per-core head-slice width
N_CORES = 8
CT = 8                   # C tiles of 128
P = 128
NG = B // NV             # 2 groups

_BUILT = None
TRACE = False
LAST_RESULTS = None


def _build():
    import concourse.bass as bass  # noqa: F401
    from concourse import bacc
    import concourse.mybir as mybir
    from concourse.tile import TileContext

    f32 = mybir.dt.float32
    fp8 = mybir.dt.float8e4
    bf16 = mybir.dt.bfloat16
    fp16 = mybir.dt.float16
    EXP = mybir.ActivationFunctionType.Exp
    MULT = mybir.AluOpType.mult
    DR = mybir.MatmulPerfMode.DoubleRow

    nc = bacc.Bacc("TRN2", target_bir_lowering=False, debug=False)

    # ---- DRAM tensors ----
    hsT = nc.dram_tensor("hsT", [C, T], fp8, kind="ExternalInput")
    refT = nc.dram_tensor("refT", [C, T], fp8, kind="ExternalInput")
    w_tp = {}
    for name in ("wq", "wk", "wqm", "wkm", "wqr", "wkr"):
        w_tp[name] = nc.dram_tensor(name, [P, CT, D2], fp8, kind="ExternalInput")
    wv_cat = nc.dram_tensor("wv_cat", [P, CT, 2 * D2], fp8, kind="ExternalInput")
    wvr = nc.dram_tensor("wvr", [P, CT, D2], fp8, kind="ExternalInput")
    w_pair = nc.dram_tensor("w_pair", [P, 2, C], fp8, kind="ExternalInput")
    wm = nc.dram_tensor("wm", [P, C], fp8, kind="ExternalInput")
    out_h = nc.dram_tensor("out_h", [T, C], fp16, kind="ExternalOutput")

    hsT_r = hsT.rearrange("(ct p) t -> p ct t", p=P)
    refT_r = refT.rearrange("(ct p) t -> p ct t", p=P)

    with TileContext(nc) as tc:
        with tc.tile_pool(name="wpool", bufs=1) as wpool, \
             tc.tile_pool(name="const", bufs=1) as cpool, \
             tc.tile_pool(name="hsref", bufs=2) as hpool, \
             tc.tile_pool(name="projs", bufs=2) as ppool, \
             tc.tile_pool(name="vtiles", bufs=2) as vpool, \
             tc.tile_pool(name="grp", bufs=2) as gpool, \
             tc.tile_pool(name="mvg", bufs=1) as mvgpool, \
             tc.tile_pool(name="akt", bufs=4) as apool, \
             tc.tile_pool(name="mvakt", bufs=4) as mpool, \
             tc.tile_pool(name="norm", bufs=3) as npool, \
             tc.tile_pool(name="outst", bufs=4) as opool, \
             tc.tile_pool(name="psS", bufs=3, space="PSUM") as psS, \
             tc.tile_pool(name="psPo", bufs=2, space="PSUM") as psPo, \
             tc.tile_pool(name="psPr", bufs=3, space="PSUM") as psPr:

            lp = nc.allow_low_precision(
                reason="fp8/bf16 tiles carry fp32-accumulated values")
            lp.__enter__()

            # ---- resident weights ----
            wt = {k: wpool.tile([P, CT, D2], fp8, tag=k, name=k) for k in w_tp}
            for k, dram in w_tp.items():
                nc.sync.dma_start(wt[k][:], dram[:])
            t_wv = wpool.tile([P, CT, 2 * D2], fp8, tag="wv_cat")
            nc.sync.dma_start(t_wv[:], wv_cat[:])
            t_wvr = wpool.tile([P, CT, D2], fp8, tag="wvr")
            nc.sync.dma_start(t_wvr[:], wvr[:])
            t_wp = wpool.tile([P, 2, C], fp8, tag="w_pair")
            nc.sync.dma_start(t_wp[:], w_pair[:])
            t_wm = wpool.tile([P, C], fp8, tag="wm")
            nc.sync.dma_start(t_wm[:], wm[:])

            # ---- constants ----
            of32 = cpool.tile([P, 64], f32, tag="of32")
            nc.any.memset(of32[:], 1.0)

            # ---- per-batch state (rotating pool tiles), keyed by batch ----
            st = {}          # i -> dict of tiles for batch i
            grp_t = {}       # g -> dict of group tiles

            def ensure_group(g):
                if g in grp_t:
                    return grp_t[g]
                d = {
                    "attn": gpool.tile([P, NV, CT, 2, P], fp8, tag="attn_pair", name="attn_pair"),
                    "amv": gpool.tile([P, NV, CT, P], fp8, tag="amv_pair", name="amv_pair"),
                    "qmg": mvgpool.tile([P, IH, NV, IW], bf16, tag="qmg", name="qmg"),
                    "kmg": mvgpool.tile([P, IH, NV, IW], bf16, tag="kmg", name="kmg"),
                    "blo": mvgpool.tile([P, HL, IH, 65], bf16, tag="blo", name="blo"),
                    "bhi": mvgpool.tile([64, HL, IH, 65], bf16, tag="bhi", name="bhi"),
                }
                # ones column for the mv value banks
                nc.vector.tensor_copy(
                    d["blo"][:, :, :, 64:65].rearrange("p a b o -> p (a b o)"),
                    of32[:, 0:HL * IH])
                nc.vector.tensor_copy(
                    d["bhi"][:, :, :, 64:65].rearrange("p a b o -> p (a b o)"),
                    of32[0:64, 0:HL * IH])
                grp_t[g] = d
                return d

            def emit_dma(i):
                g, v = i // NV, i % NV
                tok0 = i * S
                hs_t = hpool.tile([P, CT, S], fp8, tag="hs")
                nc.sync.dma_start(hs_t[:], hsT_r[:, :, tok0:tok0 + S])
                rf_t = hpool.tile([P, CT, S], fp8, tag="rf")
                nc.sync.dma_start(rf_t[:], refT_r[:, :, tok0:tok0 + S])
                st[i] = {"hs": hs_t, "rf": rf_t}

            def emit_projs(i):
                """q/k/qr/kr/qm/km projections + V projections for batch i.
                Emitted as filler work during batch i-1's attention."""
                g, v = i // NV, i % NV
                d = st[i]
                gd = ensure_group(g)
                qT = ppool.tile([P, S], fp8, tag="qT")
                kT = ppool.tile([P, S], fp8, tag="kT")
                qrT = ppool.tile([P, S], fp8, tag="qrT")
                krT = ppool.tile([P, S], fp8, tag="krT")
                v1 = vpool.tile([P, 4, HL, 2, 80], fp8, tag="v1")
                vr = vpool.tile([P, 4, HL, 2, 80], fp8, tag="vr")
                vm_nat = vpool.tile([P, CT, P], bf16, tag="vm_nat")
                d.update(qT=qT, kT=kT, qrT=qrT, krT=krT, v1=v1, vr=vr,
                         vm=vm_nat)
                # ones column at col 64 of each [*, ktp, j, ko, 80] slice
                for vv_ in (v1, vr):
                    nc.vector.tensor_copy(
                        vv_[:, :, :, :, 64:65].rearrange(
                            "p a b c o -> p (a b c o)"),
                        of32[:, 0:CT * HL])

                def tp_chunks(wname, src, cast_fn):
                    # transposed projection: out [128 d, 1024 tok] via 2
                    # chunks; LDW shared across chunks per kp-pair
                    prs = []
                    for ch in (0, 1):
                        prs.append(psPr.tile([P, 512], f32, tag="pr", name=f"pr{ch}"))
                    for kp in range(4):
                        for ch in (0, 1):
                            sl = slice(ch * 512, (ch + 1) * 512)
                            nc.tensor.matmul(prs[ch][:],
                                             wt[wname][:, 2 * kp:2 * kp + 2, :],
                                             src[:, 2 * kp:2 * kp + 2, sl],
                                             start=(kp == 0), stop=(kp == 3),
                                             perf_mode=DR)
                    for ch in (0, 1):
                        cast_fn(prs[ch], ch)

                def q_cast(dst):
                    def f(pr, ch):
                        nc.vector.tensor_copy(dst[:, ch * 512:(ch + 1) * 512],
                                              pr[:])
                    return f

                def m_cast(dstg):
                    def f(pr, ch):
                        # rows ch*16..+16 of the 32 ih-rows for view v
                        nc.vector.tensor_copy(
                            dstg[:, ch * 16:(ch + 1) * 16, v, :],
                            pr[:].rearrange("p (r c) -> p r c", c=IW))
                    return f

                tp_chunks("wq", d["hs"], q_cast(qT))
                tp_chunks("wk", d["hs"], q_cast(kT))
                tp_chunks("wqr", d["hs"], q_cast(qrT))
                tp_chunks("wkr", d["rf"], q_cast(krT))
                d["projm"] = lambda: (tp_chunks("wqm", d["hs"],
                                                m_cast(gd["qmg"])),
                                      tp_chunks("wkm", d["hs"],
                                                m_cast(gd["kmg"])))

                # ---- natural-orientation V projections ----
                for tt in range(CT):
                    tsl = slice(tt * P, (tt + 1) * P)
                    pv_ = psPr.tile([P, 512], f32, tag="pr")
                    for kp in range(4):
                        nc.tensor.matmul(pv_[:, 0:256],
                                         d["hs"][:, 2 * kp:2 * kp + 2, tsl],
                                         t_wv[:, 2 * kp:2 * kp + 2, :],
                                         start=(kp == 0), stop=(kp == 3),
                                         perf_mode=DR)
                    nc.vector.tensor_copy(
                        v1[:, tt // 2, :, tt % 2, 0:64],
                        pv_[:, 0:128].rearrange("p (j e) -> p j e", j=HL))
                    nc.vector.tensor_copy(vm_nat[:, tt, :], pv_[:, 128:256])
                    pr_ = psPr.tile([P, 512], f32, tag="pr")
                    for kp in range(4):
                        nc.tensor.matmul(pr_[:, 0:128],
                                         d["rf"][:, 2 * kp:2 * kp + 2, tsl],
                                         t_wvr[:, 2 * kp:2 * kp + 2, :],
                                         start=(kp == 0), stop=(kp == 3),
                                         perf_mode=DR)
                    nc.vector.tensor_copy(
                        vr[:, tt // 2, :, tt % 2, 0:64],
                        pr_[:, 0:128].rearrange("p (j e) -> p j e", j=HL))

                # vm_nat -> row banks (strided SBUF->SBUF DMAs)
                bnk = gd["blo"] if v < 4 else gd["bhi"]
                vv = v if v < 4 else v - 4
                for rl in range(4):
                    for hdI in range(HL):
                        nc.sync.dma_start(
                            bnk[32 * vv:32 * vv + 32, hdI, rl::4, 0:64],
                            vm_nat[rl * 32:(rl + 1) * 32, :,
                                   hdI * 64:(hdI + 1) * 64])

            def emit_attention(i, extras=()):
                g, v = i // NV, i % NV
                d = st[i]
                gd = ensure_group(g)
                ex = list(extras)
                for br, (uq, uk, uv, sub) in enumerate((
                        (d["qT"], d["kT"], d["v1"], 0),
                        (d["qrT"], d["krT"], d["vr"], 1))):
                    for qc in range(2):
                        qsl = slice(qc * 512, (qc + 1) * 512)
                        po = [psPo.tile([80, 512], f32, tag="po", name=f"po{j_}")
                              for j_ in range(HL)]
                        for ktp in range(4):
                            sc = [[psS.tile([P, 512], f32, tag="s",
                                            name=f"sc{j_}_{kk_}")
                                   for kk_ in range(2)] for j_ in range(HL)]
                            for kk in range(2):
                                kt = 2 * ktp + kk
                                for j in range(HL):
                                    hd = slice(64 * j, 64 * j + 64)
                                    nc.tensor.matmul(
                                        sc[j][kk][:],
                                        uk[hd, kt * P:(kt + 1) * P],
                                        uq[hd, qsl],
                                        start=True, stop=True)
                            aa = []
                            for j in range(HL):
                                a = apool.tile([P, 2, 512], fp8, tag="a2")
                                for kk in range(2):
                                    nc.scalar.activation(
                                        a[:, kk, :], sc[j][kk][:],
                                        EXP, scale=0.125)
                                aa.append(a)
                            for j in range(HL):
                                nc.tensor.matmul(
                                    po[j][:],
                                    uv[:, ktp, j, :, :],
                                    aa[j][:],
                                    start=(ktp == 0), stop=(ktp == 3),
                                    perf_mode=DR)
                            for _ in range(2):
                                if ex:
                                    ex.pop(0)()
                        # normalization per head
                        for j in range(HL):
                            hd = slice(64 * j, 64 * j + 64)
                            r0 = npool.tile([1, 512], f32, tag="r0")
                            nc.vector.tensor_copy(r0[:], po[j][64:65, :])
                            r1 = npool.tile([1, 512], f32, tag="r1")
                            nc.vector.reciprocal_approx_fast(
                                out=r1[:], in_=r0[:])
                            binv = npool.tile([64, 512], f32, tag="binv")
                            nc.gpsimd.partition_broadcast(
                                binv[:], r1[:], channels=64)
                            nc.vector.tensor_tensor(
                                gd["attn"][hd, v, qc * 4:(qc + 1) * 4, sub, :],
                                po[j][0:64, :].rearrange(
                                    "p (t c) -> p t c", c=P),
                                binv[:].rearrange("p (t c) -> p t c", c=P),
                                MULT)
                while ex:
                    ex.pop(0)()

            def emit_mv_step(g, rt2, j):
                gd = grp_t[g]
                hd = slice(64 * j, 64 * j + 64)
                r0 = 2 * rt2
                pl = psPr.tile([P, 512], f32, tag="pr")
                ph = psPr.tile([P, 512], f32, tag="pr")
                plv = pl[:, 0:384].rearrange("p (r c) -> p r c", r=2)
                phv = ph[0:64, 0:384].rearrange("p (r c) -> p r c", r=2)
                for rl in range(2):
                    r = r0 + rl
                    nc.tensor.matmul(plv[:, rl, :], gd["kmg"][hd, r, 0:4, :],
                                     gd["qmg"][hd, r, :, :],
                                     start=True, stop=True)
                    nc.tensor.matmul(phv[:, rl, :], gd["kmg"][hd, r, 4:6, :],
                                     gd["qmg"][hd, r, :, :],
                                     start=True, stop=True)
                al = mpool.tile([P, 2, 192], bf16, tag="al")
                ah = mpool.tile([64, 2, 192], bf16, tag="ah")
                nc.scalar.activation(al[:].rearrange("p a b -> p (a b)"),
                                     pl[:, 0:384], EXP, scale=0.125)
                nc.scalar.activation(ah[:].rearrange("p a b -> p (a b)"),
                                     ph[0:64, 0:384], EXP, scale=0.125)
                pom = psS.tile([P, 512], f32, tag="s", name="pom")
                pomv = pom[0:65, 0:384].rearrange("p (r c) -> p r c", r=2)
                for rl in range(2):
                    r = r0 + rl
                    nc.tensor.matmul(pomv[:, rl, :], gd["blo"][:, j, r, :],
                                     al[:, rl, :], start=True, stop=False)
                    nc.tensor.matmul(pomv[:, rl, :], gd["bhi"][:, j, r, :],
                                     ah[:, rl, :], start=False, stop=True)
                rm0 = npool.tile([1, 384], f32, tag="rm0")
                nc.vector.tensor_copy(rm0[:], pom[64:65, 0:384])
                rm = npool.tile([1, 384], f32, tag="rm")
                nc.vector.reciprocal_approx_fast(out=rm[:], in_=rm0[:])
                bm = npool.tile([64, 384], f32, tag="bm")
                nc.gpsimd.partition_broadcast(bm[:], rm[:], channels=64)
                # tokens: row r (2 rows) x 192 (view-major) -> amv[hd, v, rt, off]
                rt = r0 // 4
                off = (r0 % 4) * 32
                nc.vector.tensor_tensor(
                    gd["amv"][hd, :, rt, off:off + 64].rearrange(
                        "p v (r c) -> p v r c", r=2),
                    pomv[0:64, :, :].rearrange("p r (v c) -> p v r c", v=NV),
                    bm[:].rearrange("p (r v c) -> p v r c", r=2, v=NV),
                    MULT)

            def emit_outproj(g, v2, rt):
                gd = grp_t[g]
                tokb = (g * NV + v2) * S + rt * P
                ps0 = psPr.tile([P, 512], f32, tag="pr")
                ps1 = psPr.tile([P, 512], f32, tag="pr")
                stt = gd["attn"][:, v2, rt, :, :]
                nc.tensor.matmul(ps0[:], stt, t_wp[:, :, 0:512],
                                 start=True, stop=False, perf_mode=DR)
                nc.tensor.matmul(ps1[:], stt, t_wp[:, :, 512:1024],
                                 start=True, stop=False, perf_mode=DR)
                sm = gd["amv"][:, v2, rt, :]
                nc.tensor.matmul(ps0[:], sm, t_wm[:, 0:512],
                                 start=False, stop=True)
                nc.tensor.matmul(ps1[:], sm, t_wm[:, 512:1024],
                                 start=False, stop=True)
                ost = opool.tile([P, C], fp16, tag="ost")
                if v2 % 2 == 0:
                    nc.vector.tensor_copy(ost[:, 0:512], ps0[:])
                    nc.vector.tensor_copy(ost[:, 512:1024], ps1[:])
                else:
                    nc.scalar.copy(ost[:, 0:512], ps0[:])
                    nc.scalar.copy(ost[:, 512:1024], ps1[:])
                nc.sync.dma_start(out_h[tokb:tokb + P, :], ost[:])

            # =================== emission schedule ===================
            # outproj distribution over the next group's batches v=0..4
            op_sched = {0: 10, 1: 10, 2: 10, 3: 9, 4: 9}

            emit_dma(0)
            emit_projs(0)
            for i in range(B):
                g, v = i // NV, i % NV
                if i + 1 < B:
                    emit_dma(i + 1)
                # qm/km projections for batch i (into this group's mv tiles)
                st[i]["projm"]()
                if v == NV - 1:
                    # all views done -> group g's mv attention, interleaved
                    # into the attention emission so the PSUM slot rotations
                    # (and engine streams) stay mixed
                    mvx = [(lambda rj=rj: emit_mv_step(g, rj // 2, rj % 2))
                           for rj in range(32)]
                    if i == B - 1:
                        # final batch: also interleave the last group's
                        # out-projections (v2<5 becomes ready as mv streams)
                        for v2 in range(NV - 1):
                            for rt in range(CT):
                                mvx.append(
                                    lambda v2=v2, rt=rt:
                                    emit_outproj(NG - 1, v2, rt))
                    emit_attention(i, mvx)
                else:
                    emit_attention(i)
                if i + 1 < B:
                    emit_projs(i + 1)
                # deferred out-projections for the previous group
                pg = g - 1
                if pg >= 0 and v in op_sched:
                    n = op_sched[v]
                    base = sum(op_sched[k] for k in op_sched if k < v)
                    for k in range(base, base + n):
                        emit_outproj(pg, k // CT, k % CT)
                if i == B - 1:
                    for rt in range(CT):
                        emit_outproj(NG - 1, NV - 1, rt)

            lp.__exit__(None, None, None)

    nc.compile()
    return nc


def _get_built():
    global _BUILT
    if _BUILT is None:
        _BUILT = _build()
    return _BUILT


def kernel(**inputs):
    nc = _get_built()
    from concourse.bass_utils import run_bass_kernel_spmd

    fp8 = ml_dtypes.float8_e4m3fn
    hs = np.asarray(inputs["hidden_states"], np.float32)
    ref = np.asarray(inputs["ref_hidden_states"], np.float32)
    hsT = np.ascontiguousarray(hs.reshape(T, C).T).astype(fp8)
    refT = np.ascontiguousarray(ref.reshape(T, C).T).astype(fp8)

    def tp_w(w, hc):  # [C, 128] slice -> [128 Cpart, 8 Ctile, 128] fp8
        return np.ascontiguousarray(
            np.asarray(w, np.float32)[:, hc].reshape(CT, P, D2)
            .transpose(1, 0, 2)).astype(fp8)

    in_maps = []
    for c in range(N_CORES):
        hc = slice(D2 * c, D2 * (c + 1))
        wvc = np.concatenate(
            [np.asarray(inputs["Wv"], np.float32)[:, hc],
             np.asarray(inputs["Wv_mv"], np.float32)[:, hc]], axis=1)
        wp = np.stack([np.asarray(inputs["Wout"], np.float32)[hc, :],
                       np.asarray(inputs["Wout_ref"], np.float32)[hc, :]],
                      axis=1)
        in_maps.append({
            "hsT": hsT, "refT": refT,
            "wq": tp_w(inputs["Wq"], hc), "wk": tp_w(inputs["Wk"], hc),
            "wqm": tp_w(inputs["Wq_mv"], hc), "wkm": tp_w(inputs["Wk_mv"], hc),
            "wqr": tp_w(inputs["Wq_ref"], hc), "wkr": tp_w(inputs["Wk_ref"], hc),
            "wvr": tp_w(inputs["Wv_ref"], hc),
            "wv_cat": np.ascontiguousarray(
                wvc.reshape(CT, P, 2 * D2).transpose(1, 0, 2)).astype(fp8),
            "w_pair": np.ascontiguousarray(wp).astype(fp8),
            "wm": np.ascontiguousarray(
                np.asarray(inputs["Wout_mv"], np.float32)[hc, :]).astype(fp8),
        })

    global LAST_RESULTS
    kwargs = {}
    if TRACE:
        kwargs = dict(trace=True, trace_cores=list(range(N_CORES)))
    res = run_bass_kernel_spmd(nc, in_maps, core_ids=list(range(N_CORES)), **kwargs)
    LAST_RESULTS = res

    acc = np.zeros((T, C), np.float32)
    for r in res.results:
        acc += r["out_h"].astype(np.float32)
    acc += hs.reshape(T, C)
    acc += (np.asarray(inputs["bout"], np.float32)
            + np.asarray(inputs["bout_mv"], np.float32)
            + np.asarray(inputs["bout_ref"], np.float32))[None, :]
    return acc.reshape(B, S, C)
